# revision 12
# baseline (speedup 1.0000x reference)
"""HBV hydrological model (nn_HBVMulTDET_WaterLoss) as a Bass/Tile kernel on
8 Trainium2 NeuronCores.

Strategy: pure data parallelism over the 4000 grid cells (500 cells/core).
Per-core layout: partition p in [0,125) holds 4 cells x 4 components = 16
state lanes in the free dim. The T=365 recurrence is a fully unrolled
instruction stream balanced across three engines:
  - DVE: the soil-moisture critical cycle (pow via ln/exp affine, recharge,
    evap, capillary) with NZ-clamps fused into scalar_tensor_tensor ops.
  - Pool (GPSIMD): snow pack/meltwater scan + upper/lower-zone response,
    using only tensor_tensor add/sub/mult (the only TT ops Pool supports).
  - ACT: all max(x,0)-style clamps as Relu, plus Ln/Exp for the two powers.
    A single activation table (natural_log_exp_and_others, set id 6) is
    seeded explicitly so the compiler never reloads tables (1.28us each).
Algebraic restructurings vs the straight reference:
  - recharge/excess are never materialized: SUZ1 = (SUZ + SM + wi) - SMc.
  - capillary's min(.,1) and relu are provably no-ops (C<=1, SM3<=FC);
    SM4 and SLZ1 are computed from SLZ-linearized forms
    SM4 = max(SM3',NZ)*(1 - C*SLZ/FC) + C*SLZ, SLZ1 = SLZ*(1-C) + s*C*SLZ/FC
    whose SLZ-dependent factors are computed off the critical path.
  - evap factor uses SM1 instead of min(SM1,FC): identical after clip-to-1.
  - snow states are stored shifted by -NEARZERO so every clamp is a Relu.
  - x^b = exp(b*ln(x) - b*ln(FC)) with b*ln(FC) hoisted to bulk precompute.
Gamma unit-hydrograph weights are computed on host (tiny [15,4000]
preprocessing of conv_params); the routing convolution runs on device.
"""
import math
import numpy as np

T_FULL = 365
NGRID = 4000
NCORES = 8
NSH = NGRID // NCORES      # 500 cells per core
PPART = 125                # partitions used
CL = 4                     # cells per partition
M = 4                      # nmul components
LENF = 15
NZ = 1e-5
TC = 32                    # time-chunk length

# pp rows (param index in params_raw, scale, bias); K1/K2 ship separately
# pre-scaled as the packed "kk" tensor. CFR is sign-folded (negated).
PP_ROWS = [
    (0, 5.0, 1.0),       # 0 BETA
    (1, 950.0, 50.0),    # 1 FC
    (2, 0.85, 0.05),     # 2 K0
    (5, 0.8, 0.2),       # 3 LP
    (6, 10.0, 0.0),      # 4 PERC
    (7, 100.0, 0.0),     # 5 UZL
    (8, 5.0, -2.5),      # 6 TT
    (9, 9.5, 0.5),       # 7 CFMAX
    (10, -0.1, 0.0),     # 8 CFRn = -CFR
    (11, 0.2, 0.0),      # 9 CWH
    (12, 4.7, 0.3),      # 10 BETAET
    (13, 1.0, 0.0),      # 11 C
]
I_BETA, I_FC, I_K0, I_LP, I_PERC, I_UZL, I_TT, I_CFMAX, I_CFRN, I_CWH, \
    I_BETAET, I_C = range(12)


def build_program(T=T_FULL, tc_len=TC):
    import concourse.bass as bass
    import concourse.bacc as bacc
    import concourse.mybir as mybir
    import concourse.tile as tile

    F32 = mybir.dt.float32
    op = mybir.AluOpType
    AF = mybir.ActivationFunctionType

    nc = bacc.Bacc("TRN2")
    pp = nc.declare_dram_parameter("pp", [12, PPART, T, CL, M], F32, isOutput=False)
    kk = nc.declare_dram_parameter("kk", [PPART, T, 2, CL, M], F32, isOutput=False)
    xf = nc.declare_dram_parameter("xf", [3, PPART, T, CL], F32, isOutput=False)
    uh = nc.declare_dram_parameter("uh", [PPART, LENF * CL], F32, isOutput=False)
    qr = nc.declare_dram_parameter("qr", [PPART, T, CL], F32, isOutput=True)

    chunks = [(t0, min(tc_len, T - t0)) for t0 in range(0, T, tc_len)]

    with tile.TileContext(nc) as tctx:
        with (
            tctx.tile_pool(name="par", bufs=2) as par_pool,
            tctx.tile_pool(name="blk", bufs=2) as blk_pool,
            tctx.tile_pool(name="st", bufs=4) as st_pool,
            tctx.tile_pool(name="per", bufs=1) as per_pool,
        ):
            V = nc.vector
            G = nc.gpsimd
            A = nc.scalar
            S = nc.sync

            # Seed the ACT table containing BOTH Ln and Exp (and Relu/Copy):
            # natural_log_exp_and_others, set id 6. Without this the
            # table-load pass ping-pongs ln-only/exp-only tables per step.
            A.add_instruction(
                mybir.InstLoadActFuncSet(
                    name=nc.get_next_instruction_name(),
                    act_func_set_id=6, ins=[], outs=[],
                )
            )

            def tt(eng, out, a, b, o):
                eng.tensor_tensor(out, a, b, o)

            Qfull = per_pool.tile([PPART, (LENF - 1 + T) * CL], F32)
            uh_t = per_pool.tile([PPART, LENF * CL], F32)
            S.dma_start(uh_t[:], uh[:])
            G.memset(Qfull[:, : (LENF - 1) * CL], 0.0)

            state = {}
            # Snow states stored shifted by -NZ (so clamps become Relu).
            for s_, v0 in (("SPm", 0.001 - NZ), ("MWm", 0.001 - NZ),
                           ("SM", 0.001), ("SUZ", 0.001), ("SLZ", 0.001)):
                t_ = st_pool.tile([PPART, 16], F32, tag=s_)
                G.memset(t_[:], v0)
                state[s_] = t_

            def nt(tag, w=16):
                return st_pool.tile([PPART, w], F32, tag=tag, name=tag)

            pend = None     # step t-1 dict: response tail not yet emitted
            pendR = None    # step dict whose Q-reduce is not yet emitted
            snow_out = {}   # t -> wi tile (snow runs two steps ahead)
            snow_done = set()

            for (t0, tcn) in chunks:
                n16 = tcn * 16
                # ---- chunk DMAs ----
                part = {}
                for k in range(12):
                    pt = par_pool.tile([PPART, tc_len * 16], F32, tag=f"par{k}",
                                       name=f"par{k}_{t0}")
                    S.dma_start(
                        pt[:, :n16].rearrange("p (t c m) -> p t c m", c=CL, m=M),
                        pp[k, :, t0 : t0 + tcn, :, :],
                    )
                    part[k] = pt
                kkt = par_pool.tile([PPART, tc_len * 32], F32, tag="kk",
                                    name=f"kk_{t0}")
                S.dma_start(
                    kkt[:, : tcn * 32].rearrange(
                        "p (t k c m) -> p t k c m", k=2, c=CL, m=M),
                    kk[:, t0 : t0 + tcn, :, :, :],
                )
                xft = {}
                for c in range(3):
                    xt = blk_pool.tile([PPART, tc_len * CL], F32, tag=f"xf{c}",
                                       name=f"xf{c}_{t0}")
                    S.dma_start(
                        xt[:, : tcn * CL].rearrange("p (t c) -> p t c", c=CL),
                        xf[c, :, t0 : t0 + tcn, :],
                    )
                    xft[c] = xt

                # ---- parameter scaling in-place (ACT) ----
                for k, (_, sc_, bi_) in enumerate(PP_ROWS):
                    if sc_ == 1.0 and bi_ == 0.0:
                        continue
                    A.activation(part[k][:, :n16], part[k][:, :n16], AF.Copy,
                                 bias=float(bi_), scale=float(sc_))

                def bc4(xtile):
                    return (
                        xtile[:, : tcn * CL]
                        .rearrange("p (t c) -> p t c", c=CL)
                        .unsqueeze(3)
                        .to_broadcast((PPART, tcn, CL, M))
                    )

                def f4(btile):
                    return btile[:, :n16].rearrange(
                        "p (t c m) -> p t c m", c=CL, m=M
                    )

                Pb = bc4(xft[0])
                TAb = bc4(xft[1])
                PETb = bc4(xft[2])

                def bt(tag):
                    return blk_pool.tile([PPART, tc_len * 16], F32, tag=tag,
                                         name=f"{tag}_{t0}")

                # ---- bulk derived ----
                Gt = bt("Gt")
                tt(G, f4(Gt), TAb, f4(part[I_TT]), op.subtract)     # Ta - TT
                maskt = bt("mask")
                tt(V, f4(maskt), TAb, f4(part[I_TT]), op.is_ge)
                RAIN = bt("RAIN")
                tt(G, f4(RAIN), f4(maskt), Pb, op.mult)
                SNOW = bt("SNOW")
                tt(G, f4(SNOW), Pb, f4(RAIN), op.subtract)
                gc0 = bt("gc0")
                tt(G, gc0[:, :n16], part[I_CFMAX][:, :n16], Gt[:, :n16], op.mult)
                Gc = bt("Gc")
                A.activation(Gc[:, :n16], gc0[:, :n16], AF.Relu)
                CFMXn = bt("CFMXn")
                tt(G, CFMXn[:, :n16], part[I_CFRN][:, :n16],
                   part[I_CFMAX][:, :n16], op.mult)
                rc0 = bt("rc0")
                tt(G, rc0[:, :n16], CFMXn[:, :n16], Gt[:, :n16], op.mult)
                Rc = bt("Rc")
                A.activation(Rc[:, :n16], rc0[:, :n16], AF.Relu)

                FCinv = bt("FCinv")
                scr = bt("scr")
                V.reciprocal_approx_accurate(FCinv[:, :n16],
                                             part[I_FC][:, :n16],
                                             scr[:, :n16])
                CFCinv = bt("CFCinv")
                tt(V, CFCinv[:, :n16], part[I_C][:, :n16], FCinv[:, :n16],
                   op.mult)
                K1c = bt("K1c")
                kk1 = kkt[:, : tcn * 32].rearrange(
                    "p (t k x) -> p t k x", k=2, x=16)
                V.tensor_scalar(
                    K1c[:, :n16].rearrange("p (t x) -> p t x", x=16),
                    kk1[:, :, 0, :], -1.0, 1.0, op0=op.mult, op1=op.add)
                K2c = bt("K2c")
                V.tensor_scalar(
                    K2c[:, :n16].rearrange("p (t x) -> p t x", x=16),
                    kk1[:, :, 1, :], -1.0, 1.0, op0=op.mult, op1=op.add)
                lnFC = bt("lnFC")
                A.activation(lnFC[:, :n16], part[I_FC][:, :n16], AF.Ln)
                BlnFC = bt("BlnFC")
                tt(G, BlnFC[:, :n16], part[I_BETA][:, :n16], lnFC[:, :n16],
                   op.mult)
                LPFC = bt("LPFC")
                tt(G, LPFC[:, :n16], part[I_LP][:, :n16], part[I_FC][:, :n16],
                   op.mult)
                lnLPFC = bt("lnLPFC")
                A.activation(lnLPFC[:, :n16], LPFC[:, :n16], AF.Ln)
                BlnLPFC = bt("BlnLPFC")
                tt(G, BlnLPFC[:, :n16], part[I_BETAET][:, :n16],
                   lnLPFC[:, :n16], op.mult)

                def emit_snow(ti_):
                    """Snow scan for step t0+ti_ (Pool TT + ACT Relu),
                    pipelined two steps ahead of the soil cycle. States are
                    shifted by -NZ so every clamp is a Relu."""
                    snow_done.add(t0 + ti_)
                    sl_ = slice(ti_ * 16, (ti_ + 1) * 16)
                    SPm, MWm = state["SPm"], state["MWm"]
                    SP1m = nt("SP1m")
                    tt(G, SP1m[:], SPm[:], SNOW[:, sl_], op.add)
                    dd1 = nt("dd1")
                    tt(G, dd1[:], SP1m[:], Gc[:, sl_], op.subtract)
                    m1 = nt("m1")
                    A.activation(m1[:], dd1[:], AF.Relu)    # max(d1,NZ)-NZ
                    melt = nt("melt")
                    tt(G, melt[:], SP1m[:], m1[:], op.subtract)
                    MW1m = nt("MW1m")
                    tt(G, MW1m[:], MWm[:], melt[:], op.add)
                    dd2 = nt("dd2")
                    tt(G, dd2[:], MW1m[:], Rc[:, sl_], op.subtract)
                    m2 = nt("m2")
                    A.activation(m2[:], dd2[:], AF.Relu)    # MW2 - NZ
                    rfz = nt("rfz")
                    tt(G, rfz[:], MW1m[:], m2[:], op.subtract)
                    SP3m = nt("SPm")
                    tt(G, SP3m[:], m1[:], rfz[:], op.add)
                    state["SPm"] = SP3m
                    W = nt("W")
                    tt(G, W[:], part[I_CWH][:, sl_], SP3m[:], op.mult)
                    dd3 = nt("dd3")
                    tt(G, dd3[:], m2[:], W[:], op.subtract)
                    tos = nt("tos")
                    A.activation(tos[:], dd3[:], AF.Relu)
                    MW3m = nt("MWm")
                    tt(G, MW3m[:], m2[:], tos[:], op.subtract)
                    state["MWm"] = MW3m
                    wi = nt("wi")
                    tt(G, wi[:], RAIN[:, sl_], tos[:], op.add)
                    snow_out[t0 + ti_] = wi

                def emit_tail(p):
                    """Response tail of step p['t'] (Pool + one DVE relu):
                    qp -> qm -> Q0 -> SUZ3 -> SUZn, then Q1|Q2. Emitted one
                    step later; the Q-reduce is deferred one further step."""
                    qp_ = nt("qp")
                    tt(G, qp_[:], p["SUZ2"][:], p["UZL"], op.subtract)
                    qm_ = nt("qm")
                    V.tensor_scalar_max(qm_[:], qp_[:], 0.0)
                    Q048 = nt("Q048", 48)
                    tt(G, Q048[:, 0:16], qm_[:], p["K0"], op.mult)      # Q0
                    QS_ = p["QS"]   # [SUZ3 | SLZ2]; SLZ2 already written
                    tt(G, QS_[:, 0:16], p["SUZ2"][:], Q048[:, 0:16],
                       op.subtract)
                    SUZn = nt("SUZ")
                    tt(G, SUZn[:], p["K1c"], QS_[:, 0:16], op.mult)
                    state["SUZ"] = SUZn
                    tt(G, Q048[:, 16:48], p["kk2"], QS_[:], op.mult)
                    p["Q048"] = Q048

                def emit_reduce(p):
                    V.tensor_reduce(
                        Qfull[:, (LENF - 1 + p["t"]) * CL
                              : (LENF + p["t"]) * CL],
                        p["Q048"][:].rearrange(
                            "p (b c m) -> p c b m", b=3, c=CL, m=M),
                        axis=mybir.AxisListType.XY,
                        op=op.add,
                    )

                # snow for the first two steps of this chunk (not emitted by
                # earlier periods: their bulk tiles did not exist yet)
                for ti_ in range(min(2, tcn)):
                    if t0 + ti_ not in snow_done and t0 + ti_ < T:
                        emit_snow(ti_)

                # ---- sequential steps ----
                for ti in range(tcn):
                    t = t0 + ti
                    sl = slice(ti * 16, (ti + 1) * 16)
                    sl2 = slice(ti * 32, (ti + 1) * 32)

                    def ps(k):
                        return part[k][:, sl]

                    SM, SLZ = state["SM"], state["SLZ"]
                    wi = snow_out.pop(t)

                    # -- ACT: pure cycle chain --
                    lna = nt("lna")
                    A.activation(lna[:], SM[:], AF.Ln)

                    # -- Pool: tail of t-1 first (its deps are all done),
                    #    then helpers for t, then snow(t+2) --
                    if pend is not None:
                        emit_tail(pend)
                    SMa = nt("SMa")
                    tt(G, SMa[:], SM[:], wi[:], op.add)
                    SUS = nt("SUS")
                    tt(G, SUS[:], state["SUZ"][:], SMa[:], op.add)

                    # -- DVE cycle --
                    u1 = nt("u1")
                    tt(V, u1[:], ps(I_BETA), lna[:], op.mult)
                    e1 = nt("e1")
                    tt(V, e1[:], u1[:], BlnFC[:, sl], op.subtract)
                    E1 = nt("E1")
                    A.activation(E1[:], e1[:], AF.Exp)          # (SM/FC)^B
                    rech = nt("rech")
                    V.scalar_tensor_tensor(rech[:], E1[:], 1.0, wi[:],
                                           op.min, op.mult)
                    SM1 = nt("SM1")
                    tt(V, SM1[:], SMa[:], rech[:], op.subtract)
                    lnb = nt("lnb")
                    A.activation(lnb[:], SM1[:], AF.Ln)
                    # window fillers with full-period slack
                    if pendR is not None:
                        emit_reduce(pendR)
                        pendR = None
                    u2 = nt("u2")
                    tt(V, u2[:], ps(I_BETAET), lnb[:], op.mult)
                    e2 = nt("e2")
                    tt(V, e2[:], u2[:], BlnLPFC[:, sl], op.subtract)
                    E2 = nt("E2")
                    A.activation(E2[:], e2[:], AF.Exp)
                    SMc = nt("SMc")
                    tt(V, SMc[:], SM1[:], ps(I_FC), op.min)
                    # snow two steps ahead (same chunk only; next chunk's
                    # first two steps are emitted at that chunk's top)
                    if ti + 2 < tcn:
                        emit_snow(ti + 2)
                    pe = nt("pe")
                    V.scalar_tensor_tensor(
                        pe[:].rearrange("p (c m) -> p c m", m=M),
                        E2[:].rearrange("p (c m) -> p c m", m=M), 1.0,
                        PETb[:, ti, :, :], op.min, op.mult)
                    SM3p = nt("SM3p")
                    tt(V, SM3p[:], SMc[:], pe[:], op.subtract)
                    # capillary: cap = (C - max(SM3',NZ)*C/FC) * SLZ
                    v_ = nt("v")
                    V.scalar_tensor_tensor(v_[:], SM3p[:], NZ, CFCinv[:, sl],
                                           op.max, op.mult)
                    cw = nt("cw")
                    tt(V, cw[:], ps(I_C), v_[:], op.subtract)
                    cap = nt("cap")
                    tt(V, cap[:], cw[:], SLZ[:], op.mult)
                    SM4 = nt("SM")
                    V.scalar_tensor_tensor(SM4[:], SM3p[:], NZ, cap[:],
                                           op.max, op.add)
                    state["SM"] = SM4
                    SLZ1 = nt("SLZ1")
                    tt(V, SLZ1[:], SLZ[:], cap[:], op.subtract)

                    # -- response head (DVE) --
                    SUZ1 = nt("SUZ1")
                    tt(V, SUZ1[:], SUS[:], SMc[:], op.subtract)
                    PERCa = nt("PERCa")
                    tt(V, PERCa[:], SUZ1[:], ps(I_PERC), op.min)
                    QS = nt("QS", 32)     # [SUZ3 | SLZ2]; SUZ3 set in tail
                    V.scalar_tensor_tensor(QS[:, 16:32], SLZ1[:], NZ,
                                           PERCa[:], op.max, op.add)  # SLZ2
                    SLZn = nt("SLZ")
                    tt(V, SLZn[:], K2c[:, sl], QS[:, 16:32], op.mult)
                    state["SLZ"] = SLZn
                    SUZ2 = nt("SUZ2")
                    tt(V, SUZ2[:], SUZ1[:], PERCa[:], op.subtract)

                    pendR = pend
                    pend = {
                        "t": t, "SUZ2": SUZ2, "QS": QS,
                        "UZL": ps(I_UZL), "K0": ps(I_K0),
                        "K1c": K1c[:, sl], "kk2": kkt[:, sl2],
                    }

            # ---- final deferred tails + reduces ----
            if pendR is not None:
                emit_reduce(pendR)
            if pend is not None:
                emit_tail(pend)
                emit_reduce(pend)

            # ---- gamma-UH routing (DVE, bulk) ----
            Qr = per_pool.tile([PPART, T * CL], F32)
            prod = per_pool.tile([PPART, T * CL], F32)

            def qr4(ap_):
                return ap_.rearrange("p (t c) -> p t c", c=CL)

            for k in range(LENF):
                sh = Qfull[:, (LENF - 1 - k) * CL : (LENF - 1 - k + T) * CL]
                uhk = (
                    uh_t[:, k * CL : (k + 1) * CL]
                    .unsqueeze(1)
                    .to_broadcast((PPART, T, CL))
                )
                if k == 0:
                    tt(V, qr4(Qr[:]), uhk, qr4(sh), op.mult)
                else:
                    tt(V, qr4(prod[:]), uhk, qr4(sh), op.mult)
                    tt(V, qr4(Qr[:]), qr4(Qr[:]), qr4(prod[:]), op.add)

            S.dma_start(qr[:, :, :], Qr[:].rearrange("p (t c) -> p t c", c=CL))

    return nc


# ---------------- host-side packing ----------------

def pack_inputs(x_hydro_model, params_raw, conv_params_hydro):
    T = x_hydro_model.shape[0]
    f32 = np.float32
    x = np.ascontiguousarray(x_hydro_model, dtype=f32)
    xs = x.reshape(T, NCORES, PPART, CL, 3).transpose(1, 4, 2, 0, 3)

    idx = [r[0] for r in PP_ROWS]
    pr = np.ascontiguousarray(params_raw[:, :, idx, :], dtype=f32)
    prs = pr.reshape(T, NCORES, PPART, CL, 12, M).transpose(1, 4, 2, 0, 3, 5)

    # K1/K2 pre-scaled, packed [PPART, T, 2, CL, M]
    k12 = np.ascontiguousarray(params_raw[:, :, 3:5, :], dtype=np.float64)
    k12 = k12 * np.array([0.49, 0.199]).reshape(1, 1, 2, 1) + np.array(
        [0.01, 0.001]).reshape(1, 1, 2, 1)
    k12 = k12.astype(f32).reshape(T, NCORES, PPART, CL, 2, M).transpose(
        1, 2, 0, 4, 3, 5)  # [core, PPART, T, 2, CL, M]

    conv = np.asarray(conv_params_hydro, dtype=np.float64)
    a = conv[:, 0] * 2.9
    b = conv[:, 1] * 6.5
    aa = np.maximum(a, 0) + 0.1
    theta = np.maximum(b, 0) + 0.5
    tgrid = np.arange(0.5, float(LENF), dtype=np.float64)[:, None]
    lg = np.array([math.lgamma(v) for v in aa])
    w = np.exp(-lg) / theta ** aa * tgrid ** (aa - 1.0) * np.exp(-tgrid / theta)
    w = w / w.sum(0)
    UH = (w * (1.0 / M)).astype(f32)  # [LENF, NGRID], mean-over-M folded in
    uh_c = UH.reshape(LENF, NCORES, PPART, CL).transpose(1, 2, 0, 3)

    in_maps = []
    for i in range(NCORES):
        in_maps.append({
            "pp": np.ascontiguousarray(prs[i]),
            "kk": np.ascontiguousarray(k12[i]),
            "xf": np.ascontiguousarray(xs[i]),
            "uh": np.ascontiguousarray(uh_c[i]).reshape(PPART, LENF * CL),
        })
    return in_maps


def unpack_outputs(results, T):
    out = np.empty((T, NGRID), np.float32)
    for i in range(NCORES):
        q = results[i]["qr"].reshape(PPART, T, CL)
        out[:, i * NSH : (i + 1) * NSH] = q.transpose(1, 0, 2).reshape(T, NSH)
    return out


_PROG_CACHE = {}


def kernel(x_hydro_model, params_raw, conv_params_hydro):
    from concourse.bass_utils import run_bass_kernel_spmd

    T = x_hydro_model.shape[0]
    key = T
    if key not in _PROG_CACHE:
        _PROG_CACHE[key] = build_program(T=T)
    nc = _PROG_CACHE[key]
    if not nc.is_finalized():
        nc.finalize()
    in_maps = pack_inputs(x_hydro_model, params_raw, conv_params_hydro)
    res = run_bass_kernel_spmd(nc, in_maps, list(range(NCORES)))
    return unpack_outputs(res.results, T)


# revision 13
# speedup vs baseline: 1.6878x; 1.6878x over previous
"""HBV hydrological model (nn_HBVMulTDET_WaterLoss) as a Bass/Tile kernel on
8 Trainium2 NeuronCores.

Strategy: pure data parallelism over the 4000 grid cells (500 cells/core).
Per-core layout: partition p in [0,125) holds 4 cells x 4 components = 16
state lanes in the free dim. The T=365 recurrence is a fully unrolled,
manually *paced* instruction stream (tile_wait_until timestamps define a
4.6us steady-state period) balanced across three engines:
  - DVE: the soil-moisture critical cycle (pow via ln/exp affine, recharge,
    evap, capillary) + SLZ/SUZ response arithmetic, NZ-clamps fused into
    scalar_tensor_tensor ops.
  - Pool (GPSIMD): snow pack/meltwater scan (pipelined two steps ahead) +
    response tail, using only tensor_tensor add/sub/mult.
  - ACT: Ln/Exp for the two powers + snow clamps as Relu, each slotted a
    safe margin after its producers so the in-order queue never blocks the
    critical chain. One activation table (set id 6: ln+exp+relu+copy) is
    seeded explicitly so no ACT_TABLE_LOADs are ever inserted.
Algebraic restructurings vs the straight reference:
  - recharge/excess never materialized: SUZ1 = (SUZ + SM + wi) - SMc.
  - capillary's min(.,1) and relu are provably no-ops (C<=1, SM3<=FC).
  - evap factor uses SM1 instead of min(SM1,FC): identical after clip-to-1.
  - snow states stored shifted by -NEARZERO so every clamp is a Relu.
  - x^b = exp(b*ln(x) - b*ln(FC)) with b*ln(FC) hoisted to bulk precompute.
Gamma unit-hydrograph weights are computed on host; routing runs on device.
"""
import math
import numpy as np

T_FULL = 365
NGRID = 4000
NCORES = 8
NSH = NGRID // NCORES      # 500 cells per core
PPART = 125                # partitions used
CL = 4                     # cells per partition
M = 4                      # nmul components
LENF = 15
NZ = 1e-5
TC = 32                    # time-chunk length
PMS = 0.0046               # pacing period: 4.6 us per step (in ms units)
BOFF = 4.0                 # periods of prologue slack before step 0

# pp rows (param index in params_raw, scale, bias); K1/K2 ship separately
# pre-scaled as the packed "kk" tensor. CFR is sign-folded (negated).
PP_ROWS = [
    (0, 5.0, 1.0),       # 0 BETA
    (1, 950.0, 50.0),    # 1 FC
    (2, 0.85, 0.05),     # 2 K0
    (5, 0.8, 0.2),       # 3 LP
    (6, 10.0, 0.0),      # 4 PERC
    (7, 100.0, 0.0),     # 5 UZL
    (8, 5.0, -2.5),      # 6 TT
    (9, 9.5, 0.5),       # 7 CFMAX
    (10, -0.1, 0.0),     # 8 CFRn = -CFR
    (11, 0.2, 0.0),      # 9 CWH
    (12, 4.7, 0.3),      # 10 BETAET
    (13, 1.0, 0.0),      # 11 C
]
I_BETA, I_FC, I_K0, I_LP, I_PERC, I_UZL, I_TT, I_CFMAX, I_CFRN, I_CWH, \
    I_BETAET, I_C = range(12)


def build_program(T=T_FULL, tc_len=TC):
    import concourse.bass as bass
    import concourse.bacc as bacc
    import concourse.mybir as mybir
    import concourse.tile as tile

    F32 = mybir.dt.float32
    op = mybir.AluOpType
    AF = mybir.ActivationFunctionType

    nc = bacc.Bacc("TRN2")
    pp = nc.declare_dram_parameter("pp", [12, PPART, T, CL, M], F32, isOutput=False)
    kk = nc.declare_dram_parameter("kk", [PPART, T, 2, CL, M], F32, isOutput=False)
    xf = nc.declare_dram_parameter("xf", [3, PPART, T, CL], F32, isOutput=False)
    uh = nc.declare_dram_parameter("uh", [PPART, LENF * CL], F32, isOutput=False)
    qr = nc.declare_dram_parameter("qr", [PPART, T, CL], F32, isOutput=True)

    chunks = [(t0, min(tc_len, T - t0)) for t0 in range(0, T, tc_len)]

    with tile.TileContext(nc) as tctx:
        def WU(t_, off):
            """Pacing context: schedule no earlier than period t_+off."""
            return tctx.tile_wait_until(max(0.0, (BOFF + t_ + off)) * PMS)

        with (
            tctx.tile_pool(name="par", bufs=2) as par_pool,
            tctx.tile_pool(name="blk", bufs=2) as blk_pool,
            tctx.tile_pool(name="st", bufs=4) as st_pool,
            tctx.tile_pool(name="per", bufs=1) as per_pool,
        ):
            V = nc.vector
            G = nc.gpsimd
            A = nc.scalar
            S = nc.sync

            # Seed the ACT table containing Ln, Exp, Relu and Copy
            # (natural_log_exp_and_others, set id 6) so the table-load pass
            # never inserts per-activation ACT_TABLE_LOADs (1.28us each).
            A.add_instruction(
                mybir.InstLoadActFuncSet(
                    name=nc.get_next_instruction_name(),
                    act_func_set_id=6, ins=[], outs=[],
                )
            )

            def tt(eng, out, a, b, o):
                eng.tensor_tensor(out, a, b, o)

            Qfull = per_pool.tile([PPART, (LENF - 1 + T) * CL], F32)
            uh_t = per_pool.tile([PPART, LENF * CL], F32)
            S.dma_start(uh_t[:], uh[:])
            G.memset(Qfull[:, : (LENF - 1) * CL], 0.0)

            state = {}
            # Snow states stored shifted by -NZ (so clamps become Relu).
            for s_, v0 in (("SPm", 0.001 - NZ), ("MWm", 0.001 - NZ),
                           ("SM", 0.001), ("SUZ", 0.001), ("SLZ", 0.001)):
                t_ = st_pool.tile([PPART, 16], F32, tag=s_)
                G.memset(t_[:], v0)
                state[s_] = t_

            def nt(tag, w=16):
                return st_pool.tile([PPART, w], F32, tag=tag, name=tag)

            pend = None     # step t-1 dict: response tail not yet emitted
            pendR = None    # step dict whose Q-reduce is not yet emitted
            snow_out = {}   # t -> wi tile (snow runs two steps ahead)
            snow_done = set()

            for (t0, tcn) in chunks:
                n16 = tcn * 16
                # ---- chunk DMAs + bulk precompute, slotted into the idle
                #      gaps of the previous chunk's periods ----
                part = {}
                with WU(t0, -9.0):
                    for k in range(12):
                        pt = par_pool.tile([PPART, tc_len * 16], F32,
                                           tag=f"par{k}", name=f"par{k}_{t0}")
                        S.dma_start(
                            pt[:, :n16].rearrange("p (t c m) -> p t c m",
                                                  c=CL, m=M),
                            pp[k, :, t0 : t0 + tcn, :, :],
                        )
                        part[k] = pt
                    kkt = par_pool.tile([PPART, tc_len * 32], F32, tag="kk",
                                        name=f"kk_{t0}")
                    S.dma_start(
                        kkt[:, : tcn * 32].rearrange(
                            "p (t k c m) -> p t k c m", k=2, c=CL, m=M),
                        kk[:, t0 : t0 + tcn, :, :, :],
                    )
                    xft = {}
                    for c in range(3):
                        xt = blk_pool.tile([PPART, tc_len * CL], F32,
                                           tag=f"xf{c}", name=f"xf{c}_{t0}")
                        S.dma_start(
                            xt[:, : tcn * CL].rearrange("p (t c) -> p t c",
                                                        c=CL),
                            xf[c, :, t0 : t0 + tcn, :],
                        )
                        xft[c] = xt

                # parameter scaling in-place (ACT), staggered
                for k, (_, sc_, bi_) in enumerate(PP_ROWS):
                    if sc_ == 1.0 and bi_ == 0.0:
                        continue
                    with WU(t0, -6.0 + 0.25 * k):
                        A.activation(part[k][:, :n16], part[k][:, :n16],
                                     AF.Copy, bias=float(bi_),
                                     scale=float(sc_))

                def bc4(xtile):
                    return (
                        xtile[:, : tcn * CL]
                        .rearrange("p (t c) -> p t c", c=CL)
                        .unsqueeze(3)
                        .to_broadcast((PPART, tcn, CL, M))
                    )

                def f4(btile):
                    return btile[:, :n16].rearrange(
                        "p (t c m) -> p t c m", c=CL, m=M
                    )

                Pb = bc4(xft[0])
                TAb = bc4(xft[1])
                PETb = bc4(xft[2])

                def bt(tag):
                    return blk_pool.tile([PPART, tc_len * 16], F32, tag=tag,
                                         name=f"{tag}_{t0}")

                # ---- bulk derived, staggered into previous-chunk gaps ----
                Gt = bt("Gt")
                maskt = bt("mask")
                with WU(t0, -3.4):
                    tt(G, f4(Gt), TAb, f4(part[I_TT]), op.subtract)
                    tt(V, f4(maskt), TAb, f4(part[I_TT]), op.is_ge)
                RAIN = bt("RAIN")
                SNOW = bt("SNOW")
                gc0 = bt("gc0")
                CFMXn = bt("CFMXn")
                with WU(t0, -3.0):
                    tt(G, f4(RAIN), f4(maskt), Pb, op.mult)
                    tt(G, f4(SNOW), Pb, f4(RAIN), op.subtract)
                    tt(G, gc0[:, :n16], part[I_CFMAX][:, :n16], Gt[:, :n16],
                       op.mult)
                    tt(G, CFMXn[:, :n16], part[I_CFRN][:, :n16],
                       part[I_CFMAX][:, :n16], op.mult)
                rc0 = bt("rc0")
                Gc = bt("Gc")
                with WU(t0, -2.6):
                    tt(G, rc0[:, :n16], CFMXn[:, :n16], Gt[:, :n16], op.mult)
                    A.activation(Gc[:, :n16], gc0[:, :n16], AF.Relu)
                Rc = bt("Rc")
                with WU(t0, -2.2):
                    A.activation(Rc[:, :n16], rc0[:, :n16], AF.Relu)

                FCinv = bt("FCinv")
                scr = bt("scr")
                CFCinv = bt("CFCinv")
                K1c = bt("K1c")
                K2c = bt("K2c")
                kk1 = kkt[:, : tcn * 32].rearrange(
                    "p (t k x) -> p t k x", k=2, x=16)
                with WU(t0, -3.2):
                    V.reciprocal_approx_accurate(FCinv[:, :n16],
                                                 part[I_FC][:, :n16],
                                                 scr[:, :n16])
                    V.tensor_scalar(
                        K1c[:, :n16].rearrange("p (t x) -> p t x", x=16),
                        kk1[:, :, 0, :], -1.0, 1.0, op0=op.mult, op1=op.add)
                    V.tensor_scalar(
                        K2c[:, :n16].rearrange("p (t x) -> p t x", x=16),
                        kk1[:, :, 1, :], -1.0, 1.0, op0=op.mult, op1=op.add)
                with WU(t0, -2.6):
                    tt(V, CFCinv[:, :n16], part[I_C][:, :n16],
                       FCinv[:, :n16], op.mult)

                lnFC = bt("lnFC")
                LPFC = bt("LPFC")
                with WU(t0, -3.4):
                    A.activation(lnFC[:, :n16], part[I_FC][:, :n16], AF.Ln)
                    tt(G, LPFC[:, :n16], part[I_LP][:, :n16],
                       part[I_FC][:, :n16], op.mult)
                BlnFC = bt("BlnFC")
                lnLPFC = bt("lnLPFC")
                with WU(t0, -2.9):
                    tt(G, BlnFC[:, :n16], part[I_BETA][:, :n16],
                       lnFC[:, :n16], op.mult)
                    A.activation(lnLPFC[:, :n16], LPFC[:, :n16], AF.Ln)
                BlnLPFC = bt("BlnLPFC")
                with WU(t0, -2.4):
                    tt(G, BlnLPFC[:, :n16], part[I_BETAET][:, :n16],
                       lnLPFC[:, :n16], op.mult)

                def emit_snow(ti_):
                    """Snow scan for step s = t0+ti_, emitted two steps ahead
                    (during period p = s-2) and paced across periods p..p+1.
                    Pool TT only; clamps are ACT Relus slotted with a full
                    margin after their producers."""
                    s = t0 + ti_
                    snow_done.add(s)
                    p = s - 2
                    sl_ = slice(ti_ * 16, (ti_ + 1) * 16)
                    SPm, MWm = state["SPm"], state["MWm"]
                    SP1m = nt("SP1m")
                    dd1 = nt("dd1")
                    with WU(p, 0.08):
                        tt(G, SP1m[:], SPm[:], SNOW[:, sl_], op.add)
                        tt(G, dd1[:], SP1m[:], Gc[:, sl_], op.subtract)
                    m1 = nt("m1")
                    with WU(p, 0.60):
                        A.activation(m1[:], dd1[:], AF.Relu)
                    melt = nt("melt")
                    MW1m = nt("MW1m")
                    dd2 = nt("dd2")
                    with WU(p, 0.68):
                        tt(G, melt[:], SP1m[:], m1[:], op.subtract)
                        tt(G, MW1m[:], MWm[:], melt[:], op.add)
                        tt(G, dd2[:], MW1m[:], Rc[:, sl_], op.subtract)
                    m2 = nt("m2")
                    with WU(p, 0.88):
                        A.activation(m2[:], dd2[:], AF.Relu)
                    rfz = nt("rfz")
                    SP3m = nt("SPm")
                    W = nt("W")
                    dd3 = nt("dd3")
                    with WU(p, 0.95):
                        tt(G, rfz[:], MW1m[:], m2[:], op.subtract)
                        tt(G, SP3m[:], m1[:], rfz[:], op.add)
                        tt(G, W[:], part[I_CWH][:, sl_], SP3m[:], op.mult)
                        tt(G, dd3[:], m2[:], W[:], op.subtract)
                    state["SPm"] = SP3m
                    tos = nt("tos")
                    with WU(p, 1.18):
                        A.activation(tos[:], dd3[:], AF.Relu)
                    MW3m = nt("MWm")
                    wi = nt("wi")
                    with WU(p, 1.24):
                        tt(G, MW3m[:], m2[:], tos[:], op.subtract)
                        tt(G, wi[:], RAIN[:, sl_], tos[:], op.add)
                    state["MWm"] = MW3m
                    snow_out[s] = wi

                def emit_tail(p_, t_):
                    """Response tail of step p_['t'], paced into period t_."""
                    qp_ = nt("qp")
                    with WU(t_, 0.0):
                        tt(G, qp_[:], p_["SUZ2"][:], p_["UZL"], op.subtract)
                    qm_ = nt("qm")
                    with WU(t_, 0.12):
                        V.tensor_scalar_max(qm_[:], qp_[:], 0.0)
                    Q048 = nt("Q048", 48)
                    QS_ = p_["QS"]   # [SUZ3 | SLZ2]; SLZ2 already written
                    with WU(t_, 0.20):
                        tt(G, Q048[:, 0:16], qm_[:], p_["K0"], op.mult)
                        tt(G, QS_[:, 0:16], p_["SUZ2"][:], Q048[:, 0:16],
                           op.subtract)
                    SUZn = nt("SUZ")
                    with WU(t_, 0.28):
                        tt(G, SUZn[:], p_["K1c"], QS_[:, 0:16], op.mult)
                        tt(G, Q048[:, 16:48], p_["kk2"], QS_[:], op.mult)
                    state["SUZ"] = SUZn
                    p_["Q048"] = Q048

                def emit_reduce(p_, t_):
                    with WU(t_, 0.45):
                        V.tensor_reduce(
                            Qfull[:, (LENF - 1 + p_["t"]) * CL
                                  : (LENF + p_["t"]) * CL],
                            p_["Q048"][:].rearrange(
                                "p (b c m) -> p c b m", b=3, c=CL, m=M),
                            axis=mybir.AxisListType.XY,
                            op=op.add,
                        )

                # snow for the first two steps of this chunk
                for ti_ in range(min(2, tcn)):
                    if t0 + ti_ not in snow_done and t0 + ti_ < T:
                        emit_snow(ti_)

                # ---- sequential steps ----
                for ti in range(tcn):
                    t = t0 + ti
                    sl = slice(ti * 16, (ti + 1) * 16)
                    sl2 = slice(ti * 32, (ti + 1) * 32)

                    def ps(k):
                        return part[k][:, sl]

                    SM, SLZ = state["SM"], state["SLZ"]
                    wi = snow_out.pop(t)

                    if pend is not None:
                        emit_tail(pend, t)
                    SMa = nt("SMa")
                    with WU(t, 0.04):
                        tt(G, SMa[:], SM[:], wi[:], op.add)
                    SUS = nt("SUS")
                    with WU(t, 0.34):
                        tt(G, SUS[:], state["SUZ"][:], SMa[:], op.add)

                    # -- critical cycle (ACT + DVE), dep-driven within the
                    #    period --
                    with WU(t, 0.0):
                        lna = nt("lna")
                        A.activation(lna[:], SM[:], AF.Ln)
                        u1 = nt("u1")
                        tt(V, u1[:], ps(I_BETA), lna[:], op.mult)
                        e1 = nt("e1")
                        tt(V, e1[:], u1[:], BlnFC[:, sl], op.subtract)
                        E1 = nt("E1")
                        A.activation(E1[:], e1[:], AF.Exp)
                        rech = nt("rech")
                        V.scalar_tensor_tensor(rech[:], E1[:], 1.0, wi[:],
                                               op.min, op.mult)
                        SM1 = nt("SM1")
                        tt(V, SM1[:], SMa[:], rech[:], op.subtract)
                        lnb = nt("lnb")
                        A.activation(lnb[:], SM1[:], AF.Ln)
                        u2 = nt("u2")
                        tt(V, u2[:], ps(I_BETAET), lnb[:], op.mult)
                        e2 = nt("e2")
                        tt(V, e2[:], u2[:], BlnLPFC[:, sl], op.subtract)
                        E2 = nt("E2")
                        A.activation(E2[:], e2[:], AF.Exp)
                        SMc = nt("SMc")
                        tt(V, SMc[:], SM1[:], ps(I_FC), op.min)
                        pe = nt("pe")
                        V.scalar_tensor_tensor(
                            pe[:].rearrange("p (c m) -> p c m", m=M),
                            E2[:].rearrange("p (c m) -> p c m", m=M), 1.0,
                            PETb[:, ti, :, :], op.min, op.mult)
                        SM3p = nt("SM3p")
                        tt(V, SM3p[:], SMc[:], pe[:], op.subtract)
                        # capillary: cap = (C - max(SM3',NZ)*C/FC) * SLZ
                        v_ = nt("v")
                        V.scalar_tensor_tensor(v_[:], SM3p[:], NZ,
                                               CFCinv[:, sl], op.max, op.mult)
                        cw = nt("cw")
                        tt(V, cw[:], ps(I_C), v_[:], op.subtract)
                        cap = nt("cap")
                        tt(V, cap[:], cw[:], SLZ[:], op.mult)
                        SM4 = nt("SM")
                        V.scalar_tensor_tensor(SM4[:], SM3p[:], NZ, cap[:],
                                               op.max, op.add)
                        state["SM"] = SM4

                    # -- deferred reduce of step t-2 --
                    if pendR is not None:
                        emit_reduce(pendR, t)
                        pendR = None

                    # -- response head (DVE), slotted after SUS/cap --
                    SUZ1 = nt("SUZ1")
                    with WU(t, 0.50):
                        tt(V, SUZ1[:], SUS[:], SMc[:], op.subtract)
                    PERCa = nt("PERCa")
                    with WU(t, 0.56):
                        tt(V, PERCa[:], SUZ1[:], ps(I_PERC), op.min)
                    SUZ2 = nt("SUZ2")
                    with WU(t, 0.62):
                        tt(V, SUZ2[:], SUZ1[:], PERCa[:], op.subtract)
                    SLZ1 = nt("SLZ1")
                    QS = nt("QS", 32)     # [SUZ3 | SLZ2]; SUZ3 set in tail
                    with WU(t, 0.84):
                        tt(V, SLZ1[:], SLZ[:], cap[:], op.subtract)
                    with WU(t, 0.90):
                        V.scalar_tensor_tensor(QS[:, 16:32], SLZ1[:], NZ,
                                               PERCa[:], op.max, op.add)
                    SLZn = nt("SLZ")
                    with WU(t, 0.96):
                        tt(V, SLZn[:], K2c[:, sl], QS[:, 16:32], op.mult)
                    state["SLZ"] = SLZn

                    # snow two steps ahead (same chunk only)
                    if ti + 2 < tcn:
                        emit_snow(ti + 2)

                    pendR = pend
                    pend = {
                        "t": t, "SUZ2": SUZ2, "QS": QS,
                        "UZL": ps(I_UZL), "K0": ps(I_K0),
                        "K1c": K1c[:, sl], "kk2": kkt[:, sl2],
                    }

            # ---- final deferred tails + reduces ----
            TE = T + 1
            if pendR is not None:
                emit_reduce(pendR, TE)
            if pend is not None:
                emit_tail(pend, TE)
                emit_reduce(pend, TE + 1)

            # ---- gamma-UH routing (DVE, bulk) ----
            with WU(TE + 2, 0.0):
                Qr = per_pool.tile([PPART, T * CL], F32)
                prod = per_pool.tile([PPART, T * CL], F32)

                def qr4(ap_):
                    return ap_.rearrange("p (t c) -> p t c", c=CL)

                for k in range(LENF):
                    sh = Qfull[:, (LENF - 1 - k) * CL
                               : (LENF - 1 - k + T) * CL]
                    uhk = (
                        uh_t[:, k * CL : (k + 1) * CL]
                        .unsqueeze(1)
                        .to_broadcast((PPART, T, CL))
                    )
                    if k == 0:
                        tt(V, qr4(Qr[:]), uhk, qr4(sh), op.mult)
                    else:
                        tt(V, qr4(prod[:]), uhk, qr4(sh), op.mult)
                        tt(V, qr4(Qr[:]), qr4(Qr[:]), qr4(prod[:]), op.add)

                S.dma_start(qr[:, :, :],
                            Qr[:].rearrange("p (t c) -> p t c", c=CL))

    return nc


# ---------------- host-side packing ----------------

def pack_inputs(x_hydro_model, params_raw, conv_params_hydro):
    T = x_hydro_model.shape[0]
    f32 = np.float32
    x = np.ascontiguousarray(x_hydro_model, dtype=f32)
    xs = x.reshape(T, NCORES, PPART, CL, 3).transpose(1, 4, 2, 0, 3)

    idx = [r[0] for r in PP_ROWS]
    pr = np.ascontiguousarray(params_raw[:, :, idx, :], dtype=f32)
    prs = pr.reshape(T, NCORES, PPART, CL, 12, M).transpose(1, 4, 2, 0, 3, 5)

    # K1/K2 pre-scaled, packed [PPART, T, 2, CL, M]
    k12 = np.ascontiguousarray(params_raw[:, :, 3:5, :], dtype=np.float64)
    k12 = k12 * np.array([0.49, 0.199]).reshape(1, 1, 2, 1) + np.array(
        [0.01, 0.001]).reshape(1, 1, 2, 1)
    k12 = k12.astype(f32).reshape(T, NCORES, PPART, CL, 2, M).transpose(
        1, 2, 0, 4, 3, 5)  # [core, PPART, T, 2, CL, M]

    conv = np.asarray(conv_params_hydro, dtype=np.float64)
    a = conv[:, 0] * 2.9
    b = conv[:, 1] * 6.5
    aa = np.maximum(a, 0) + 0.1
    theta = np.maximum(b, 0) + 0.5
    tgrid = np.arange(0.5, float(LENF), dtype=np.float64)[:, None]
    lg = np.array([math.lgamma(v) for v in aa])
    w = np.exp(-lg) / theta ** aa * tgrid ** (aa - 1.0) * np.exp(-tgrid / theta)
    w = w / w.sum(0)
    UH = (w * (1.0 / M)).astype(f32)  # [LENF, NGRID], mean-over-M folded in
    uh_c = UH.reshape(LENF, NCORES, PPART, CL).transpose(1, 2, 0, 3)

    in_maps = []
    for i in range(NCORES):
        in_maps.append({
            "pp": np.ascontiguousarray(prs[i]),
            "kk": np.ascontiguousarray(k12[i]),
            "xf": np.ascontiguousarray(xs[i]),
            "uh": np.ascontiguousarray(uh_c[i]).reshape(PPART, LENF * CL),
        })
    return in_maps


def unpack_outputs(results, T):
    out = np.empty((T, NGRID), np.float32)
    for i in range(NCORES):
        q = results[i]["qr"].reshape(PPART, T, CL)
        out[:, i * NSH : (i + 1) * NSH] = q.transpose(1, 0, 2).reshape(T, NSH)
    return out


_PROG_CACHE = {}


def kernel(x_hydro_model, params_raw, conv_params_hydro):
    from concourse.bass_utils import run_bass_kernel_spmd

    T = x_hydro_model.shape[0]
    key = T
    if key not in _PROG_CACHE:
        _PROG_CACHE[key] = build_program(T=T)
    nc = _PROG_CACHE[key]
    if not nc.is_finalized():
        nc.finalize()
    in_maps = pack_inputs(x_hydro_model, params_raw, conv_params_hydro)
    res = run_bass_kernel_spmd(nc, in_maps, list(range(NCORES)))
    return unpack_outputs(res.results, T)


# revision 23
# speedup vs baseline: 1.7683x; 1.0477x over previous
"""HBV hydrological model (nn_HBVMulTDET_WaterLoss) as a Bass/Tile kernel on
8 Trainium2 NeuronCores.

Strategy: pure data parallelism over the 4000 grid cells (500 cells/core).
Per-core layout: partition p in [0,125) holds 4 cells x 4 components = 16
state lanes in the free dim. The T=365 recurrence is a fully unrolled,
manually *paced* instruction stream (tile_wait_until timestamps define a
4.6us steady-state period) balanced across three engines:
  - DVE: the soil-moisture critical cycle (pow via ln/exp affine, recharge,
    evap, capillary) + SLZ/SUZ response arithmetic, NZ-clamps fused into
    scalar_tensor_tensor ops.
  - Pool (GPSIMD): snow pack/meltwater scan (pipelined two steps ahead) +
    response tail, using only tensor_tensor add/sub/mult.
  - ACT: Ln/Exp for the two powers + snow clamps as Relu, each slotted a
    safe margin after its producers so the in-order queue never blocks the
    critical chain. One activation table (set id 6: ln+exp+relu+copy) is
    seeded explicitly so no ACT_TABLE_LOADs are ever inserted.
Algebraic restructurings vs the straight reference:
  - recharge/excess never materialized: SUZ1 = (SUZ + SM + wi) - SMc.
  - capillary's min(.,1) and relu are provably no-ops (C<=1, SM3<=FC).
  - evap factor uses SM1 instead of min(SM1,FC): identical after clip-to-1.
  - snow states stored shifted by -NEARZERO so every clamp is a Relu.
  - x^b = exp(b*ln(x) - b*ln(FC)) with b*ln(FC) hoisted to bulk precompute.
Gamma unit-hydrograph weights are computed on host; routing runs on device.
"""
import math
import numpy as np

T_FULL = 365
NGRID = 4000
NCORES = 8
NSH = NGRID // NCORES      # 500 cells per core
PPART = 125                # partitions used
CL = 4                     # cells per partition
M = 4                      # nmul components
LENF = 15
NZ = 1e-5
TC = 32                    # time-chunk length
PMS = 0.0050               # pacing period: 5.0 us per step (in ms units)
BOFF = 4.0                 # periods of prologue slack before step 0

# pp rows, pre-scaled on host: (param index, scale, bias). CFR is
# sign-folded (negated). Rows 11/12 are host-computed B*ln(FC) and
# BETAET*ln(LP*FC). K1/K2 ship separately as the packed "kk" tensor.
PP_ROWS = [
    (0, 5.0, 1.0),       # 0 BETA
    (1, 950.0, 50.0),    # 1 FC
    (2, 0.85, 0.05),     # 2 K0
    (6, 10.0, 0.0),      # 3 PERC
    (7, 100.0, 0.0),     # 4 UZL
    (8, 5.0, -2.5),      # 5 TT
    (9, 9.5, 0.5),       # 6 CFMAX
    (10, -0.1, 0.0),     # 7 CFRn = -CFR
    (11, 0.2, 0.0),      # 8 CWH
    (13, 1.0, 0.0),      # 9 C
    (12, 4.7, 0.3),      # 10 BETAET (kept for u2)
]
NPP = 13                   # 11 scaled rows + BlnFC + BlnLPFC
I_BETA, I_FC, I_K0, I_PERC, I_UZL, I_TT, I_CFMAX, I_CFRN, I_CWH, \
    I_C, I_BETAET = range(11)
I_BLNFC, I_BLNLPFC = 11, 12


def build_program(T=T_FULL, tc_len=TC):
    import concourse.bass as bass
    import concourse.bacc as bacc
    import concourse.mybir as mybir
    import concourse.tile as tile

    F32 = mybir.dt.float32
    op = mybir.AluOpType
    AF = mybir.ActivationFunctionType

    nc = bacc.Bacc("TRN2")
    pp = nc.declare_dram_parameter("pp", [NPP, PPART, T, CL, M], F32, isOutput=False)
    kk = nc.declare_dram_parameter("kk", [PPART, T, 2, CL, M], F32, isOutput=False)
    xf = nc.declare_dram_parameter("xf", [3, PPART, T, CL], F32, isOutput=False)
    uh = nc.declare_dram_parameter("uh", [PPART, LENF * CL], F32, isOutput=False)
    qr = nc.declare_dram_parameter("qr", [PPART, T, CL], F32, isOutput=True)

    chunks = [(t0, min(tc_len, T - t0)) for t0 in range(0, T, tc_len)]

    with tile.TileContext(nc) as tctx:
        def WU(t_, off):
            """Pacing context: schedule no earlier than period t_+off."""
            return tctx.tile_wait_until(max(0.0, (BOFF + t_ + off)) * PMS)

        with (
            tctx.tile_pool(name="par", bufs=2) as par_pool,
            tctx.tile_pool(name="blk", bufs=2) as blk_pool,
            tctx.tile_pool(name="st", bufs=4) as st_pool,
            tctx.tile_pool(name="per", bufs=1) as per_pool,
        ):
            V = nc.vector
            G = nc.gpsimd
            A = nc.scalar
            S = nc.sync

            # Seed the ACT table containing Ln, Exp, Relu and Copy
            # (natural_log_exp_and_others, set id 6) so the table-load pass
            # never inserts per-activation ACT_TABLE_LOADs (1.28us each).
            A.add_instruction(
                mybir.InstLoadActFuncSet(
                    name=nc.get_next_instruction_name(),
                    act_func_set_id=6, ins=[], outs=[],
                )
            )

            def tt(eng, out, a, b, o):
                eng.tensor_tensor(out, a, b, o)

            Qfull = per_pool.tile([PPART, (LENF - 1 + T) * CL], F32)
            uh_t = per_pool.tile([PPART, LENF * CL], F32)
            S.dma_start(uh_t[:], uh[:])
            G.memset(Qfull[:, : (LENF - 1) * CL], 0.0)

            state = {}
            # Snow states stored shifted by -NZ (so clamps become Relu).
            for s_, v0 in (("SPm", 0.001 - NZ), ("MWm", 0.001 - NZ),
                           ("SM", 0.001), ("SUZ", 0.001), ("SLZ", 0.001)):
                t_ = st_pool.tile([PPART, 16], F32, tag=s_)
                G.memset(t_[:], v0)
                state[s_] = t_

            def nt(tag, w=16):
                return st_pool.tile([PPART, w], F32, tag=tag, name=tag)

            pend = None     # step t-1 dict: response tail not yet emitted
            pendR = None    # step dict whose Q-reduce is not yet emitted
            snow_out = {}   # t -> wi tile (snow runs two steps ahead)
            snow_done = set()

            for (t0, tcn) in chunks:
                n16 = tcn * 16
                # ---- chunk DMAs + bulk precompute, slotted into the idle
                #      gaps of the previous chunk's periods ----
                part = {}
                with WU(t0, -9.0):
                    for k in range(NPP):
                        pt = par_pool.tile([PPART, tc_len * 16], F32,
                                           tag=f"par{k}", name=f"par{k}_{t0}")
                        S.dma_start(
                            pt[:, :n16].rearrange("p (t c m) -> p t c m",
                                                  c=CL, m=M),
                            pp[k, :, t0 : t0 + tcn, :, :],
                        )
                        part[k] = pt
                    kkt = par_pool.tile([PPART, tc_len * 32], F32, tag="kk",
                                        name=f"kk_{t0}")
                    S.dma_start(
                        kkt[:, : tcn * 32].rearrange(
                            "p (t k c m) -> p t k c m", k=2, c=CL, m=M),
                        kk[:, t0 : t0 + tcn, :, :, :],
                    )
                    xft = {}
                    for c in range(3):
                        xt = blk_pool.tile([PPART, tc_len * CL], F32,
                                           tag=f"xf{c}", name=f"xf{c}_{t0}")
                        S.dma_start(
                            xt[:, : tcn * CL].rearrange("p (t c) -> p t c",
                                                        c=CL),
                            xf[c, :, t0 : t0 + tcn, :],
                        )
                        xft[c] = xt

                def bc4(xtile):
                    return (
                        xtile[:, : tcn * CL]
                        .rearrange("p (t c) -> p t c", c=CL)
                        .unsqueeze(3)
                        .to_broadcast((PPART, tcn, CL, M))
                    )

                def f4(btile):
                    return btile[:, :n16].rearrange(
                        "p (t c m) -> p t c m", c=CL, m=M
                    )

                Pb = bc4(xft[0])
                TAb = bc4(xft[1])
                PETb = bc4(xft[2])

                def bt(tag):
                    return blk_pool.tile([PPART, tc_len * 16], F32, tag=tag,
                                         name=f"{tag}_{t0}")

                # ---- bulk derived, staggered into previous-chunk gaps ----
                Gt = bt("Gt")
                maskt = bt("mask")
                with WU(t0, -3.4):
                    tt(G, f4(Gt), TAb, f4(part[I_TT]), op.subtract)
                    tt(V, f4(maskt), TAb, f4(part[I_TT]), op.is_ge)
                RAIN = bt("RAIN")
                SNOW = bt("SNOW")
                gc0 = bt("gc0")
                CFMXn = bt("CFMXn")
                with WU(t0, -3.0):
                    tt(G, f4(RAIN), f4(maskt), Pb, op.mult)
                    tt(G, f4(SNOW), Pb, f4(RAIN), op.subtract)
                    tt(G, gc0[:, :n16], part[I_CFMAX][:, :n16], Gt[:, :n16],
                       op.mult)
                    tt(G, CFMXn[:, :n16], part[I_CFRN][:, :n16],
                       part[I_CFMAX][:, :n16], op.mult)
                rc0 = bt("rc0")
                Gc = bt("Gc")
                with WU(t0, -2.6):
                    tt(G, rc0[:, :n16], CFMXn[:, :n16], Gt[:, :n16], op.mult)
                    A.activation(Gc[:, :n16], gc0[:, :n16], AF.Relu)
                Rc = bt("Rc")
                with WU(t0, -2.2):
                    A.activation(Rc[:, :n16], rc0[:, :n16], AF.Relu)

                FCinv = bt("FCinv")
                scr = bt("scr")
                CFCinv = bt("CFCinv")
                K1c = bt("K1c")
                K2c = bt("K2c")
                kk1 = kkt[:, : tcn * 32].rearrange(
                    "p (t k x) -> p t k x", k=2, x=16)
                with WU(t0, -3.2):
                    V.reciprocal_approx_accurate(FCinv[:, :n16],
                                                 part[I_FC][:, :n16],
                                                 scr[:, :n16])
                    V.tensor_scalar(
                        K1c[:, :n16].rearrange("p (t x) -> p t x", x=16),
                        kk1[:, :, 0, :], -1.0, 1.0, op0=op.mult, op1=op.add)
                    V.tensor_scalar(
                        K2c[:, :n16].rearrange("p (t x) -> p t x", x=16),
                        kk1[:, :, 1, :], -1.0, 1.0, op0=op.mult, op1=op.add)
                with WU(t0, -2.6):
                    tt(V, CFCinv[:, :n16], part[I_C][:, :n16],
                       FCinv[:, :n16], op.mult)
                BlnFC = part[I_BLNFC]
                BlnLPFC = part[I_BLNLPFC]

                def emit_snow(ti_):
                    """Snow scan for step s = t0+ti_, emitted two steps ahead
                    (during period p = s-2) and paced across periods p..p+1.
                    Pool TT only; clamps are ACT Relus slotted with a full
                    margin after their producers."""
                    s = t0 + ti_
                    snow_done.add(s)
                    p = s - 2
                    sl_ = slice(ti_ * 16, (ti_ + 1) * 16)
                    SPm, MWm = state["SPm"], state["MWm"]
                    SP1m = nt("SP1m")
                    dd1 = nt("dd1")
                    with WU(p, 0.08):
                        tt(G, SP1m[:], SPm[:], SNOW[:, sl_], op.add)
                        tt(G, dd1[:], SP1m[:], Gc[:, sl_], op.subtract)
                    m1 = nt("m1")
                    with WU(p, 0.60):
                        A.activation(m1[:], dd1[:], AF.Relu)
                    melt = nt("melt")
                    MW1m = nt("MW1m")
                    dd2 = nt("dd2")
                    with WU(p, 0.68):
                        tt(G, melt[:], SP1m[:], m1[:], op.subtract)
                        tt(G, MW1m[:], MWm[:], melt[:], op.add)
                        tt(G, dd2[:], MW1m[:], Rc[:, sl_], op.subtract)
                    m2 = nt("m2")
                    with WU(p, 0.88):
                        A.activation(m2[:], dd2[:], AF.Relu)
                    rfz = nt("rfz")
                    SP3m = nt("SPm")
                    W = nt("W")
                    dd3 = nt("dd3")
                    with WU(p, 0.95):
                        tt(G, rfz[:], MW1m[:], m2[:], op.subtract)
                        tt(G, SP3m[:], m1[:], rfz[:], op.add)
                        tt(G, W[:], part[I_CWH][:, sl_], SP3m[:], op.mult)
                        tt(G, dd3[:], m2[:], W[:], op.subtract)
                    state["SPm"] = SP3m
                    tos = nt("tos")
                    with WU(p, 1.18):
                        A.activation(tos[:], dd3[:], AF.Relu)
                    MW3m = nt("MWm")
                    wi = nt("wi")
                    with WU(p, 1.24):
                        tt(G, MW3m[:], m2[:], tos[:], op.subtract)
                        tt(G, wi[:], RAIN[:, sl_], tos[:], op.add)
                    state["MWm"] = MW3m
                    snow_out[s] = wi

                def emit_tail(p_, t_):
                    """Response tail of step p_['t'], paced into period t_."""
                    qp_ = nt("qp")
                    with WU(t_, 0.0):
                        tt(G, qp_[:], p_["SUZ2"][:], p_["UZL"], op.subtract)
                    qm_ = nt("qm")
                    with WU(t_, 0.30):
                        A.activation(qm_[:], qp_[:], AF.Relu)
                    Q048 = nt("Q048", 48)
                    QS_ = p_["QS"]   # [SUZ3 | SLZ2]; SLZ2 already written
                    with WU(t_, 0.38):
                        tt(G, Q048[:, 0:16], qm_[:], p_["K0"], op.mult)
                        tt(G, QS_[:, 0:16], p_["SUZ2"][:], Q048[:, 0:16],
                           op.subtract)
                    SUZn = nt("SUZ")
                    with WU(t_, 0.46):
                        tt(G, SUZn[:], p_["K1c"], QS_[:, 0:16], op.mult)
                        tt(G, Q048[:, 16:48], p_["kk2"], QS_[:], op.mult)
                    state["SUZ"] = SUZn
                    p_["Q048"] = Q048

                def emit_reduce(p_, t_):
                    with WU(t_, 0.45):
                        V.tensor_reduce(
                            Qfull[:, (LENF - 1 + p_["t"]) * CL
                                  : (LENF + p_["t"]) * CL],
                            p_["Q048"][:].rearrange(
                                "p (b c m) -> p c b m", b=3, c=CL, m=M),
                            axis=mybir.AxisListType.XY,
                            op=op.add,
                        )

                # snow for the first two steps of this chunk
                for ti_ in range(min(2, tcn)):
                    if t0 + ti_ not in snow_done and t0 + ti_ < T:
                        emit_snow(ti_)

                # ---- sequential steps ----
                for ti in range(tcn):
                    t = t0 + ti
                    sl = slice(ti * 16, (ti + 1) * 16)
                    sl2 = slice(ti * 32, (ti + 1) * 32)

                    def ps(k):
                        return part[k][:, sl]

                    SM, SLZ = state["SM"], state["SLZ"]
                    wi = snow_out.pop(t)

                    if pend is not None:
                        emit_tail(pend, t)
                    SMa = nt("SMa")
                    with WU(t, 0.04):
                        tt(G, SMa[:], SM[:], wi[:], op.add)
                    SUS = nt("SUS")
                    with WU(t, 0.54):
                        tt(V, SUS[:], state["SUZ"][:], SMa[:], op.add)

                    # -- critical cycle (ACT + DVE), dep-driven within the
                    #    period --
                    with WU(t, 0.0):
                        lna = nt("lna")
                        A.activation(lna[:], SM[:], AF.Ln)
                        u1 = nt("u1")
                        tt(V, u1[:], ps(I_BETA), lna[:], op.mult)
                        e1 = nt("e1")
                        tt(V, e1[:], u1[:], BlnFC[:, sl], op.subtract)
                        E1 = nt("E1")
                        A.activation(E1[:], e1[:], AF.Exp)
                        rech = nt("rech")
                        V.scalar_tensor_tensor(rech[:], E1[:], 1.0, wi[:],
                                               op.min, op.mult)
                        SM1 = nt("SM1")
                        tt(V, SM1[:], SMa[:], rech[:], op.subtract)
                        lnb = nt("lnb")
                        A.activation(lnb[:], SM1[:], AF.Ln)
                        u2 = nt("u2")
                        tt(V, u2[:], ps(I_BETAET), lnb[:], op.mult)
                        e2 = nt("e2")
                        tt(V, e2[:], u2[:], BlnLPFC[:, sl], op.subtract)
                        E2 = nt("E2")
                        A.activation(E2[:], e2[:], AF.Exp)
                        SMc = nt("SMc")
                        tt(V, SMc[:], SM1[:], ps(I_FC), op.min)
                        pe = nt("pe")
                        V.scalar_tensor_tensor(
                            pe[:].rearrange("p (c m) -> p c m", m=M),
                            E2[:].rearrange("p (c m) -> p c m", m=M), 1.0,
                            PETb[:, ti, :, :], op.min, op.mult)
                        SM3p = nt("SM3p")
                        tt(V, SM3p[:], SMc[:], pe[:], op.subtract)
                        # capillary: cap = (C - max(SM3',NZ)*C/FC) * SLZ
                        v_ = nt("v")
                        V.scalar_tensor_tensor(v_[:], SM3p[:], NZ,
                                               CFCinv[:, sl], op.max, op.mult)
                        cw = nt("cw")
                        tt(V, cw[:], ps(I_C), v_[:], op.subtract)
                        cap = nt("cap")
                        tt(V, cap[:], cw[:], SLZ[:], op.mult)
                        SM4 = nt("SM")
                        V.scalar_tensor_tensor(SM4[:], SM3p[:], NZ, cap[:],
                                               op.max, op.add)
                        state["SM"] = SM4

                    # -- deferred reduce of step t-2 --
                    if pendR is not None:
                        emit_reduce(pendR, t)
                        pendR = None

                    # -- response head (DVE), slotted after SUS/cap --
                    SUZ1 = nt("SUZ1")
                    with WU(t, 0.60):
                        tt(V, SUZ1[:], SUS[:], SMc[:], op.subtract)
                    PERCa = nt("PERCa")
                    with WU(t, 0.66):
                        tt(V, PERCa[:], SUZ1[:], ps(I_PERC), op.min)
                    SUZ2 = nt("SUZ2")
                    with WU(t, 0.72):
                        tt(V, SUZ2[:], SUZ1[:], PERCa[:], op.subtract)
                    SLZ1 = nt("SLZ1")
                    QS = nt("QS", 32)     # [SUZ3 | SLZ2]; SUZ3 set in tail
                    with WU(t, 0.84):
                        tt(V, SLZ1[:], SLZ[:], cap[:], op.subtract)
                    with WU(t, 0.90):
                        V.scalar_tensor_tensor(QS[:, 16:32], SLZ1[:], NZ,
                                               PERCa[:], op.max, op.add)
                    SLZn = nt("SLZ")
                    with WU(t, 0.96):
                        tt(V, SLZn[:], K2c[:, sl], QS[:, 16:32], op.mult)
                    state["SLZ"] = SLZn

                    # snow two steps ahead (same chunk only)
                    if ti + 2 < tcn:
                        emit_snow(ti + 2)

                    pendR = pend
                    pend = {
                        "t": t, "SUZ2": SUZ2, "QS": QS,
                        "UZL": ps(I_UZL), "K0": ps(I_K0),
                        "K1c": K1c[:, sl], "kk2": kkt[:, sl2],
                    }

            # ---- final deferred tails + reduces ----
            TE = T + 1
            if pendR is not None:
                emit_reduce(pendR, TE)
            if pend is not None:
                emit_tail(pend, TE)
                emit_reduce(pend, TE + 1)

            # ---- gamma-UH routing (DVE, bulk) ----
            with WU(TE + 2, 0.0):
                Qr = per_pool.tile([PPART, T * CL], F32)
                prod = per_pool.tile([PPART, T * CL], F32)

                def qr4(ap_):
                    return ap_.rearrange("p (t c) -> p t c", c=CL)

                for k in range(LENF):
                    sh = Qfull[:, (LENF - 1 - k) * CL
                               : (LENF - 1 - k + T) * CL]
                    uhk = (
                        uh_t[:, k * CL : (k + 1) * CL]
                        .unsqueeze(1)
                        .to_broadcast((PPART, T, CL))
                    )
                    if k == 0:
                        tt(V, qr4(Qr[:]), uhk, qr4(sh), op.mult)
                    else:
                        tt(V, qr4(prod[:]), uhk, qr4(sh), op.mult)
                        tt(V, qr4(Qr[:]), qr4(Qr[:]), qr4(prod[:]), op.add)

                S.dma_start(qr[:, :, :],
                            Qr[:].rearrange("p (t c) -> p t c", c=CL))

    return nc


# ---------------- host-side packing ----------------

def pack_inputs(x_hydro_model, params_raw, conv_params_hydro):
    T = x_hydro_model.shape[0]
    f32 = np.float32
    x = np.ascontiguousarray(x_hydro_model, dtype=f32)
    xs = x.reshape(T, NCORES, PPART, CL, 3).transpose(1, 4, 2, 0, 3)

    # pre-scale the 11 direct rows on host, then append host-computed
    # BlnFC = BETA*ln(FC) and BlnLPFC = BETAET*ln(LP*FC)
    idx = [r[0] for r in PP_ROWS]
    sc = np.array([r[1] for r in PP_ROWS], np.float32).reshape(1, 1, -1, 1)
    bi = np.array([r[2] for r in PP_ROWS], np.float32).reshape(1, 1, -1, 1)
    pr = params_raw[:, :, idx, :].astype(f32) * sc + bi
    beta_s = pr[:, :, 0, :]
    fc_s = pr[:, :, 1, :]
    betaet_s = pr[:, :, 10, :]
    lp_s = (params_raw[:, :, 5, :].astype(f32) * 0.8 + 0.2)
    lnfc = np.log(fc_s)
    blnfc = (beta_s * lnfc)[:, :, None, :]
    blnlpfc = (betaet_s * (np.log(lp_s) + lnfc))[:, :, None, :]
    pr = np.concatenate([pr, blnfc, blnlpfc], axis=2)
    pr = np.ascontiguousarray(pr, dtype=f32)
    prs = pr.reshape(T, NCORES, PPART, CL, NPP, M).transpose(1, 4, 2, 0, 3, 5)

    # K1/K2 pre-scaled, packed [PPART, T, 2, CL, M]
    k12 = np.ascontiguousarray(params_raw[:, :, 3:5, :], dtype=np.float64)
    k12 = k12 * np.array([0.49, 0.199]).reshape(1, 1, 2, 1) + np.array(
        [0.01, 0.001]).reshape(1, 1, 2, 1)
    k12 = k12.astype(f32).reshape(T, NCORES, PPART, CL, 2, M).transpose(
        1, 2, 0, 4, 3, 5)  # [core, PPART, T, 2, CL, M]

    conv = np.asarray(conv_params_hydro, dtype=np.float64)
    a = conv[:, 0] * 2.9
    b = conv[:, 1] * 6.5
    aa = np.maximum(a, 0) + 0.1
    theta = np.maximum(b, 0) + 0.5
    tgrid = np.arange(0.5, float(LENF), dtype=np.float64)[:, None]
    lg = np.array([math.lgamma(v) for v in aa])
    w = np.exp(-lg) / theta ** aa * tgrid ** (aa - 1.0) * np.exp(-tgrid / theta)
    w = w / w.sum(0)
    UH = (w * (1.0 / M)).astype(f32)  # [LENF, NGRID], mean-over-M folded in
    uh_c = UH.reshape(LENF, NCORES, PPART, CL).transpose(1, 2, 0, 3)

    in_maps = []
    for i in range(NCORES):
        in_maps.append({
            "pp": np.ascontiguousarray(prs[i]),
            "kk": np.ascontiguousarray(k12[i]),
            "xf": np.ascontiguousarray(xs[i]),
            "uh": np.ascontiguousarray(uh_c[i]).reshape(PPART, LENF * CL),
        })
    return in_maps


def unpack_outputs(results, T):
    out = np.empty((T, NGRID), np.float32)
    for i in range(NCORES):
        q = results[i]["qr"].reshape(PPART, T, CL)
        out[:, i * NSH : (i + 1) * NSH] = q.transpose(1, 0, 2).reshape(T, NSH)
    return out


_PROG_CACHE = {}


def kernel(x_hydro_model, params_raw, conv_params_hydro):
    from concourse.bass_utils import run_bass_kernel_spmd

    T = x_hydro_model.shape[0]
    key = T
    if key not in _PROG_CACHE:
        _PROG_CACHE[key] = build_program(T=T)
    nc = _PROG_CACHE[key]
    if not nc.is_finalized():
        nc.finalize()
    in_maps = pack_inputs(x_hydro_model, params_raw, conv_params_hydro)
    res = run_bass_kernel_spmd(nc, in_maps, list(range(NCORES)))
    return unpack_outputs(res.results, T)
